# revision 21
# baseline (speedup 1.0000x reference)
"""AO layer kernel for Trainium2 (8 NeuronCores, data-parallel over walkers).

Math: out[b,n,a] = ang(a, r) * rad(a, r),  r = pos[b,n] - centers[a]
  rad = sum_p coeffs[a,p] * exp(-exps[a,p] * |r|^2)
  ang = prod_c r_c^powers[a,c],  powers in {0,1,2}

Device formulation (per core, i = flattened (b,n) walker-electron index):
  Basis rows [xh,xl,1,0, sqh,sql, sqh,xh,1, 0...] of pos (bf16 hi/lo split;
  squares built on device), zero-padded to k=128 — full-k matmuls keep the
  PE HAM activity monitor warm (small-k matmuls leave the PE at 1.2GHz).
  z[(a,p), i]   = W1^T R    (PE bf16 matmuls; lhsT [Wh;Wh;Wl] exact split)
  E = exp(z)                (ScalarE, bf16 out)
  rad[a, i]     = S^T E     (PE bf16 matmuls, +-1 sign matrix)
  p_c[a, i]     = Q_c^T R   (PE bf16 matmuls, per-axis angular polys)
  out[a, i]     = px*py*pz*rad   (VectorE, f32r out)
  out[i, a]     = PE f32r transpose, copy, DMA to DRAM.
"""

import numpy as np
import ml_dtypes

B, NEL, A, P = 512, 32, 256, 6
NCORES = 8
BS = B // NCORES          # 64 walkers per core
I = BS * NEL              # 2048 (b,n) pairs per core
ITILE = 512
NIT = I // ITILE          # 4 i-tiles
RT = (A * P) // 128       # 12 r-tiles of 128 (a,p) rows
K7 = 7
K21 = 21

_CACHE = {}


def _bf(v):
    return np.asarray(v, np.float64).astype(ml_dtypes.bfloat16)


def _split21(w):
    """bf16 hi/lo split of [7, C] weights, laid out to match the device
    basis rows [xh,xl,1,0, sqh,sql, sqh2,xh2,1] and zero-padded to k=128
    (full-k matmuls keep the PE HAM activity monitor warm)."""
    wh = _bf(w).astype(np.float64)
    wl = _bf(np.asarray(w, np.float64) - wh)
    wh = _bf(wh)
    z1 = np.zeros((1, w.shape[1]), ml_dtypes.bfloat16)
    out = np.concatenate([
        wh[3:6], wh[3:6], wh[6:7], z1,        # xh, xl, 1, 0
        wh[0:3], wh[0:3],                     # sqh, sql
        wl[0:3], wl[3:6], wl[6:7],            # sqh2, xh2, 1
        np.zeros((128 - 21, w.shape[1]), ml_dtypes.bfloat16),
    ], axis=0)
    return np.ascontiguousarray(out)


def _build_nc():
    import concourse.bass as bass
    import concourse.bacc as bacc
    import concourse.tile as tile
    import concourse.mybir as mybir
    from concourse import masks

    f32 = mybir.dt.float32
    f32r = mybir.dt.float32r
    bf16 = mybir.dt.bfloat16
    EXP = mybir.ActivationFunctionType.Exp
    PSUM = bass.MemorySpace.PSUM

    nc = bacc.Bacc("TRN2", target_bir_lowering=False, debug=False,
                   num_devices=NCORES)

    posf_d = nc.declare_dram_parameter("posF", [3, I], f32, isOutput=False)
    posb_d = nc.declare_dram_parameter("posB", [8, I], bf16, isOutput=False)
    w1_d = nc.declare_dram_parameter("w1", [128, RT * 128], bf16, isOutput=False)
    q_d = nc.declare_dram_parameter("qq", [128, 3 * A], bf16, isOutput=False)
    s_d = nc.declare_dram_parameter("s", [128, RT * 128], bf16, isOutput=False)
    out_d = nc.declare_dram_parameter("out", [I, A], f32, isOutput=True)

    with tile.TileContext(nc) as tc:
        with (
            tc.tile_pool(name="const", bufs=1) as const,
            tc.tile_pool(name="zp", bufs=2, space=PSUM) as zp,
            tc.tile_pool(name="radp", bufs=2, space=PSUM) as radp,
            tc.tile_pool(name="scr", bufs=2, space=PSUM) as scr,
            tc.tile_pool(name="ep", bufs=3) as ep,
            tc.tile_pool(name="angp", bufs=4) as angp,
            tc.tile_pool(name="mid", bufs=3) as mid,
            tc.tile_pool(name="op", bufs=3) as op,
            tc.tile_pool(name="tp", bufs=3) as tp,
        ):
            r21 = const.tile([128, I], bf16)
            pf = const.tile([3, I], f32)
            sqf = const.tile([3, I], f32)
            sqh3 = const.tile([3, I], bf16)
            sql3 = const.tile([3, I], bf16)
            w1_sb = const.tile([128, RT * 128], bf16)
            q_sb = const.tile([128, 3 * A], bf16)
            s_sb = const.tile([128, RT * 128], bf16)
            ident = const.tile([128, 128], f32)
            identr = const.tile([128, 128], f32r)

            # --- basis rows: [xh,xl,1,0, sqh,sql,sqh2,xh2,1, 0-pad...] ---
            nc.gpsimd.memset(r21[:], 0.0)
            nc.sync.dma_start(pf[:], posf_d[:])
            nc.sync.dma_start(r21[0:8, :], posb_d[:])
            nc.vector.tensor_mul(sqf[:], pf[:], pf[:])
            nc.vector.tensor_copy(sqh3[:], sqf[:])
            nc.vector.tensor_sub(sql3[:], sqf[:], sqh3[:])
            nc.sync.dma_start(r21[8:11, :], sqh3[:])
            nc.sync.dma_start(r21[14:17, :], sqh3[:])
            nc.sync.dma_start(r21[11:14, :], sql3[:])
            nc.sync.dma_start(r21[17:20, :], posb_d[0:3, :])
            nc.sync.dma_start(r21[20:21, :], posb_d[6:7, :])

            nc.sync.dma_start(w1_sb[:], w1_d[:])
            nc.sync.dma_start(q_sb[:], q_d[:])
            nc.sync.dma_start(s_sb[:], s_d[:])
            masks.make_identity(nc, ident[:])
            nc.vector.tensor_copy(identr[:], ident[:])

            def mm(out_ap, lhs_ap, rhs_ap, start=True, stop=True):
                nc.tensor.matmul(out_ap, lhs_ap, rhs_ap, start=start, stop=stop)

            for it in range(NIT):
                i0 = it * ITILE
                ri = r21[:, i0:i0 + ITILE]

                # ---- angular: p_c = Q_c^T R, ang = px*py*pz ----
                ang = []
                for at in range(2):
                    def qs(c):
                        a0 = c * A + at * 128
                        return q_sb[:, a0:a0 + 128]
                    px = scr.tile([128, ITILE], f32, tag="scr")
                    mm(px[:], qs(0), ri)
                    py = scr.tile([128, ITILE], f32, tag="scr")
                    mm(py[:], qs(1), ri)
                    pxs = mid.tile([128, ITILE], f32, tag="pxs")
                    nc.vector.tensor_copy(pxs[:], px[:])
                    t1 = mid.tile([128, ITILE], f32, tag="t1")
                    nc.vector.tensor_mul(t1[:], pxs[:], py[:])
                    pz = scr.tile([128, ITILE], f32, tag="scr")
                    mm(pz[:], qs(2), ri)
                    a_sb = angp.tile([128, ITILE], f32, tag="ang")
                    nc.vector.tensor_mul(a_sb[:], t1[:], pz[:])
                    ang.append(a_sb)

                # ---- radial: z pairs, E=exp(z) bf16, rad = S^T E ----
                rad = []
                for _ in range(2):
                    rad_t = radp.tile([128, ITILE], f32, tag="rad")
                    rad.append(rad_t)
                for pair in range(RT // 2):
                    rts = (2 * pair, 2 * pair + 1)
                    z2 = zp.tile([128, 2 * ITILE], f32, tag="z")
                    for j, rt in enumerate(rts):
                        mm(z2[:, j * ITILE:(j + 1) * ITILE],
                           w1_sb[:, rt * 128:(rt + 1) * 128], ri)
                    e2 = ep.tile([128, 2 * ITILE], bf16, tag="e")
                    nc.scalar.activation(e2[:], z2[:], EXP)
                    for j, rt in enumerate(rts):
                        at = 0 if rt < 6 else 1
                        mm(rad[at][:], s_sb[:, rt * 128:(rt + 1) * 128],
                           e2[:, j * ITILE:(j + 1) * ITILE],
                           start=(rt % 6 == 0), stop=(rt % 6 == 5))

                # ---- final: out = ang * rad (f32r), transpose, DMA out ----
                osb = []
                for at in range(2):
                    o = op.tile([128, ITILE], f32r, tag="o")
                    nc.vector.tensor_mul(o[:], ang[at][:], rad[at][:])
                    osb.append(o)
                for blk in range(ITILE // 128):
                    tps = scr.tile([128, 2 * 128], f32r, tag="scr")
                    for at in range(2):
                        nc.tensor.transpose(
                            tps[:, at * 128:(at + 1) * 128],
                            osb[at][:, blk * 128:(blk + 1) * 128], identr[:])
                    t_sb = tp.tile([128, 2 * 128], f32, tag="tsb")
                    nc.any.tensor_copy(t_sb[:], tps[:])
                    ib = i0 + blk * 128
                    nc.sync.dma_start(out_d[ib:ib + 128, :], t_sb[:])

    nc.compile()
    return nc


def _consts(centers, exps, coeffs, powers):
    al = exps.astype(np.float64)
    c = coeffs.astype(np.float64)
    cen = centers.astype(np.float64)
    cc = (cen ** 2).sum(-1)
    absc = np.abs(c)
    lnc = np.where(absc > 0, np.log(np.where(absc > 0, absc, 1.0)), -1e30)
    sgn = np.sign(c)

    alf = al.reshape(-1)  # row index r = a*P + p
    w1 = np.zeros((K7, A * P))
    w1[0] = w1[1] = w1[2] = -alf
    for cd in range(3):
        w1[3 + cd] = 2.0 * alf * np.repeat(cen[:, cd], P)
    w1[6] = -alf * np.repeat(cc, P) + lnc.reshape(-1)
    w1 = np.float32(w1)

    s = np.zeros((RT, 128, 128))
    r = np.arange(A * P)
    t_of_r = r // 128
    m_of_r = (r // P) - np.where(t_of_r < RT // 2, 0, 128)
    s[t_of_r, r % 128, m_of_r] = sgn.reshape(-1)
    s2 = np.ascontiguousarray(s.transpose(1, 0, 2).reshape(128, RT * 128))

    qmat = np.zeros((3, K7, A))
    for cd in range(3):
        l = powers[:, cd].astype(np.int64)
        ccd = cen[:, cd]
        qmat[cd, cd] = (l == 2) * 1.0
        qmat[cd, 3 + cd] = (l == 1) * 1.0 + (l == 2) * (-2.0 * ccd)
        qmat[cd, 6] = (l == 0) * 1.0 + (l == 1) * (-ccd) + (l == 2) * (ccd ** 2)
    q2 = np.ascontiguousarray(
        np.float32(qmat).transpose(1, 0, 2).reshape(K7, 3 * A))

    return (_split21(w1), _bf(s2), _split21(q2))


LAST_RESULT = None


def kernel(pos, centers, exps, coeffs, powers):
    global LAST_RESULT
    from concourse.bass_utils import run_bass_kernel_spmd

    pos = np.asarray(pos, dtype=np.float32)
    centers = np.asarray(centers, dtype=np.float32)
    exps = np.asarray(exps, dtype=np.float32)
    coeffs = np.asarray(coeffs, dtype=np.float32)
    powers = np.asarray(powers)

    if "nc" not in _CACHE:
        _CACHE["nc"] = _build_nc()
    nc = _CACHE["nc"]

    w1, s, q = _consts(centers, exps, coeffs, powers)
    in_maps = []
    for ci in range(NCORES):
        shard = pos[ci * BS:(ci + 1) * BS].reshape(I, 3)
        posf = np.ascontiguousarray(shard.T)                  # [3, I] f32
        xh = _bf(posf)
        xl = _bf(posf.astype(np.float64) - xh.astype(np.float64))
        posb = np.concatenate(
            [xh, xl,
             np.ones((1, I), ml_dtypes.bfloat16),
             np.zeros((1, I), ml_dtypes.bfloat16)], axis=0)   # [8, I] bf16
        in_maps.append({"posF": posf, "posB": np.ascontiguousarray(posb),
                        "w1": w1, "s": s, "qq": q})

    res = run_bass_kernel_spmd(nc, in_maps, core_ids=list(range(NCORES)))
    LAST_RESULT = res
    out = np.concatenate(
        [res.results[ci]["out"].reshape(BS, NEL, A) for ci in range(NCORES)],
        axis=0)
    return out
